# revision 8
# baseline (speedup 1.0000x reference)
"""Multi-head causal attention (B=8, S=1024, E=1024, H=16, HS=64) on 8 TRN2 cores.

Strategy: pure data parallel over batch -- each NeuronCore computes one batch
element end to end (QKV projections, causal attention, output projection);
no collectives. All matmuls run as float32r (full-rate fp32) with layouts
chosen so no on-chip transposes are needed:
  - host feeds x[b].T, per-head-flattened W, Wproj.T
  - Q^T, K^T computed as [E, S] (heads stacked along partitions)
  - scores computed transposed: S^T[t, s] = K_h^T(t-block) x Q_h^T
  - softmax: exp on ACT; row sums fused into the AV matmul via a ones
    column appended to V; normalization via PE outer-product broadcast
  - V computed in natural [S, H*HS] layout; O^T[e, s] comes out of AV
    directly in the layout the output projection needs as stationary.
"""

import sys

sys.path.insert(0, "/opt/trn_rl_repo")

import numpy as np

B, S, E = 8, 1024, 1024
H, HS = 16, 64
N_CORES = 8
P = 128  # partitions
CH = 512  # matmul free-dim chunk
ET = E // P  # 8 e-tiles
ST_ = S // P  # 8 s/t-tiles
NCH = S // CH  # 2 s-chunks

_cached = {}


def _build(reps=1):
    import concourse.mybir as mybir
    import concourse.tile as tile
    from concourse import bacc

    F32 = mybir.dt.float32
    F32R = mybir.dt.float32r
    EXP = mybir.ActivationFunctionType.Exp

    nc = bacc.Bacc("TRN2", target_bir_lowering=False, debug=False, num_devices=N_CORES)

    xT = nc.dram_tensor("xT", [E, S], F32R, kind="ExternalInput").ap()
    wq = nc.dram_tensor("wq", [E, E], F32R, kind="ExternalInput").ap()
    wk = nc.dram_tensor("wk", [E, E], F32R, kind="ExternalInput").ap()
    wv = nc.dram_tensor("wv", [E, E], F32R, kind="ExternalInput").ap()
    wpT = nc.dram_tensor("wpT", [E, E], F32R, kind="ExternalInput").ap()
    bias = nc.dram_tensor("bias", [1, E], F32R, kind="ExternalInput").ap()
    mask = nc.dram_tensor("mask", [P, P], F32R, kind="ExternalInput").ap()
    ones1 = nc.dram_tensor("ones1", [1, HS], F32R, kind="ExternalInput").ap()
    ones128 = nc.dram_tensor("ones128", [1, P], F32R, kind="ExternalInput").ap()
    onescols = nc.dram_tensor("onescols", [P, H], F32R, kind="ExternalInput").ap()
    out = nc.dram_tensor("out", [S, E], F32, kind="ExternalOutput").ap()

    VW = HS + 1  # V columns per head incl ones column

    with tile.TileContext(nc) as tc:
      for _rep in range(reps):
        with (
            tc.tile_pool(name="qt_pool", bufs=1) as qtp,
            tc.tile_pool(name="kt_pool", bufs=1) as ktp,
            tc.tile_pool(name="va_pool", bufs=1) as vap,
            tc.tile_pool(name="ot_pool", bufs=1) as otp,
            tc.tile_pool(name="const_pool", bufs=1) as cp,
        ):
            qt = [qtp.tile([P, S], F32R, name=f"qt{m}") for m in range(ET)]
            kt = [ktp.tile([P, S], F32R, name=f"kt{m}") for m in range(ET)]
            vaug = [vap.tile([P, H * VW], F32R, name=f"vaug{t}") for t in range(ST_)]
            ot = [otp.tile([P, S], F32R, name=f"ot{m}") for m in range(ET)]
            mask_t = cp.tile([P, P], F32R, name="mask_t")
            ones1_t = cp.tile([1, HS], F32R, name="ones1_t")
            ones128_t = cp.tile([1, P], F32R, name="ones128_t")
            bias_t = cp.tile([1, E], F32R, name="bias_t")
            nc.sync.dma_start(mask_t[:], mask[:])
            nc.sync.dma_start(ones1_t[:], ones1[:])
            nc.sync.dma_start(ones128_t[:], ones128[:])
            nc.sync.dma_start(bias_t[:], bias[:])

            # ---------------- Phase 1: Q^T, K^T, V ----------------
            with (
                tc.tile_pool(name="xT_pool", bufs=1) as xtp,
                tc.tile_pool(name="w_pool", bufs=12) as wp,
                tc.tile_pool(name="ps1", bufs=4, space="PSUM") as ps1,
            ):
                xt = [xtp.tile([P, S], F32R, name=f"xt{e}") for e in range(ET)]
                for e in range(ET):
                    nc.sync.dma_start(xt[e][:], xT[e * P:(e + 1) * P, :])

                # Q^T and K^T: [E, S], heads stacked along partition rows
                for w_ap, dst, pfx in ((wq, qt, "q"), (wk, kt, "k")):
                    for m in range(ET):
                        wt = [
                            wp.tile([P, P], F32R, tag="w", name=f"w{pfx}{m}_{e}")
                            for e in range(ET)
                        ]
                        for e in range(ET):
                            nc.sync.dma_start(
                                wt[e][:], w_ap[e * P:(e + 1) * P, m * P:(m + 1) * P]
                            )
                        for c in range(NCH):
                            ps = ps1.tile([P, CH], F32, tag="p1", name=f"ps{pfx}{m}_{c}")
                            for e in range(ET):
                                nc.tensor.matmul(
                                    ps[:],
                                    wt[e][:],
                                    xt[e][:, c * CH:(c + 1) * CH],
                                    start=(e == 0),
                                    stop=(e == ET - 1),
                                )
                            nc.vector.tensor_copy(dst[m][:, c * CH:(c + 1) * CH], ps[:])

                # ones columns of V_aug (DMA'd from host const; f32r memset
                # fails ISA codegen)
                for t in range(ST_):
                    onesdst = vaug[t].rearrange("p (h d) -> p h d", d=VW)[:, :, HS:VW]
                    nc.sync.dma_start(onesdst, onescols[:].unsqueeze(2))

                # V natural [S, H*HS], scattered into vaug with stride VW
                for t in range(ST_):
                    wvt = [
                        wp.tile([P, CH], F32R, tag="wv", name=f"wv{t}_{c}_{e}")
                        for c in range(NCH)
                        for e in range(ET)
                    ]
                    for c in range(NCH):
                        for e in range(ET):
                            nc.sync.dma_start(
                                wvt[c * ET + e][:],
                                wv[e * P:(e + 1) * P, c * CH:(c + 1) * CH],
                            )
                        ps = ps1.tile([P, CH], F32, tag="p1", name=f"psv{t}_{c}")
                        for e in range(ET):
                            nc.tensor.matmul(
                                ps[:],
                                xt[e][:, t * P:(t + 1) * P],
                                wvt[c * ET + e][:],
                                start=(e == 0),
                                stop=(e == ET - 1),
                            )
                        nheads = CH // HS  # heads covered by this chunk
                        src = ps.rearrange("p (h d) -> p h d", d=HS)
                        dstcols = vaug[t][:, c * nheads * VW:(c + 1) * nheads * VW]
                        dst3 = dstcols.rearrange("p (h d) -> p h d", d=VW)[:, :, 0:HS]
                        nc.vector.tensor_copy(dst3, src)

            # ---------------- Phase 2+3: scores^T, softmax, AV ----------------
            with (
                tc.tile_pool(name="a_pool", bufs=6) as apl,
                tc.tile_pool(name="r_pool", bufs=4) as rpl,
                tc.tile_pool(name="st_ps", bufs=3, space="PSUM") as stps,
                tc.tile_pool(name="av_ps", bufs=2, space="PSUM") as avps,
                tc.tile_pool(name="bc_ps", bufs=2, space="PSUM") as bcps,
            ):
                for j in range(NCH):
                    s0 = j * CH
                    for h in range(H):
                        mt = h // 2
                        hoff = (h % 2) * HS
                        n_t = (s0 + CH) // P  # t-tiles 0 .. n_t-1
                        av = avps.tile([VW, CH], F32, tag="av", name=f"av{j}_{h}")
                        for i in range(n_t):
                            # cols [0, pfx) of this block have s < t: causally
                            # masked. Never compute or read them.
                            pfx = i * P - s0
                            vs = max(pfx, 0)
                            st = stps.tile([P, CH], F32, tag="st", name=f"st{j}_{h}_{i}")
                            nc.tensor.matmul(
                                st[:, vs:CH],
                                kt[mt][hoff:hoff + HS, i * P:(i + 1) * P],
                                qt[mt][hoff:hoff + HS, s0 + vs:s0 + CH],
                                start=True,
                                stop=True,
                            )
                            at = apl.tile([P, CH], F32R, tag="a", name=f"a{j}_{h}_{i}")
                            nc.scalar.activation(
                                at[:, vs:CH], st[:, vs:CH], EXP, scale=float(HS) ** -0.5
                            )
                            if 0 <= pfx <= CH - P:
                                # diagonal block: keep only s >= t
                                nc.vector.tensor_mul(
                                    at[:, pfx:pfx + P], at[:, pfx:pfx + P], mask_t[:]
                                )
                            nc.tensor.matmul(
                                av[:, vs:CH],
                                vaug[i][:, h * VW:(h + 1) * VW],
                                at[:, vs:CH],
                                start=(i == 0),
                                stop=(i == n_t - 1),
                            )
                        # normalize: row VW-1 of av is the softmax denominator
                        rinv = rpl.tile([1, CH], F32R, tag="r", name=f"r{j}_{h}")
                        with nc.allow_low_precision(reason="f32r feeds f32r matmul"):
                            nc.vector.reciprocal(rinv[:], av[HS:VW, :])
                        bc = bcps.tile([HS, CH], F32, tag="bc", name=f"bc{j}_{h}")
                        nc.tensor.matmul(bc[:], ones1_t[:], rinv[:], start=True, stop=True)
                        rb = rpl.tile([HS, CH], F32, tag="rb", name=f"rb{j}_{h}")
                        nc.vector.tensor_copy(rb[:], bc[:])
                        nc.vector.tensor_mul(
                            ot[mt][hoff:hoff + HS, s0:s0 + CH], av[0:HS, :], rb[:]
                        )

            # ---------------- Phase 4: output projection ----------------
            with (
                tc.tile_pool(name="wp_pool", bufs=1) as wpp,
                tc.tile_pool(name="fo_pool", bufs=4) as fop,
                tc.tile_pool(name="ps4", bufs=4, space="PSUM") as ps4,
            ):
                wpt = [wpp.tile([P, E], F32R, name=f"wpt{e}") for e in range(ET)]
                for e in range(ET):
                    nc.sync.dma_start(wpt[e][:], wpT[e * P:(e + 1) * P, :])
                for m in range(ST_):
                    for c in range(NCH):
                        ps = ps4.tile([P, CH], F32, tag="p4", name=f"pso{m}_{c}")
                        for e in range(ET):
                            nc.tensor.matmul(
                                ps[:],
                                ot[e][:, m * P:(m + 1) * P],
                                wpt[e][:, c * CH:(c + 1) * CH],
                                start=(e == 0),
                                stop=False,
                            )
                        nc.tensor.matmul(
                            ps[:],
                            ones128_t[:],
                            bias_t[:, c * CH:(c + 1) * CH],
                            start=False,
                            stop=True,
                        )
                        fo = fop.tile([P, CH], F32, tag="fo", name=f"fo{m}_{c}")
                        nc.vector.tensor_copy(fo[:], ps[:])
                        nc.sync.dma_start(out[m * P:(m + 1) * P, c * CH:(c + 1) * CH], fo[:])

    nc.compile()
    return nc


def _get_nc():
    if "nc" not in _cached:
        _cached["nc"] = _build()
    return _cached["nc"]


def kernel(x, Wq, Wk, Wv, Wproj, bproj):
    from concourse.bass_utils import run_bass_kernel_spmd

    nc = _get_nc()

    x = np.ascontiguousarray(x, dtype=np.float32)
    wq_f = np.ascontiguousarray(Wq.transpose(1, 0, 2).reshape(E, E), dtype=np.float32)
    wk_f = np.ascontiguousarray(Wk.transpose(1, 0, 2).reshape(E, E), dtype=np.float32)
    wv_f = np.ascontiguousarray(Wv.transpose(1, 0, 2).reshape(E, E), dtype=np.float32)
    wpT = np.ascontiguousarray(np.asarray(Wproj).T, dtype=np.float32)
    bias = np.ascontiguousarray(np.asarray(bproj).reshape(1, E), dtype=np.float32)
    mask = np.triu(np.ones((P, P), dtype=np.float32))  # [t, s]: 1 where s >= t
    ones1 = np.ones((1, HS), dtype=np.float32)
    ones128 = np.ones((1, P), dtype=np.float32)

    shared = {
        "wq": wq_f, "wk": wk_f, "wv": wv_f, "wpT": wpT, "bias": bias,
        "mask": mask, "ones1": ones1, "ones128": ones128,
        "onescols": np.ones((P, H), dtype=np.float32),
    }
    in_maps = [
        {"xT": np.ascontiguousarray(x[b].T), **shared} for b in range(N_CORES)
    ]
    res = run_bass_kernel_spmd(nc, in_maps, core_ids=list(range(N_CORES)))
    return np.stack([res.results[b]["out"] for b in range(B)], axis=0)


# revision 10
# speedup vs baseline: 1.2363x; 1.2363x over previous
"""Multi-head causal attention (B=8, S=1024, E=1024, H=16, HS=64) on 8 TRN2 cores.

Strategy: pure data parallel over batch -- each NeuronCore computes one batch
element end to end (QKV projections, causal attention, output projection);
no collectives. Matmul operands are float32r (fast-path fp32, ~2 cyc/row) or
bfloat16 (1 cyc/row) per MODE, with layouts chosen so no on-chip transposes
are needed:
  - host feeds x[b].T, per-head-flattened W, Wproj.T, pre-broadcast bias
  - Q^T, K^T computed as [E, S] (heads stacked along partitions)
  - scores computed transposed: S^T[t, s] = K_h^T(t-block) x Q_h^T
  - causally-masked column ranges are never computed (PSUM subrange accum)
  - softmax: exp on ACT; row sums fused into the AV matmul via a ones
    column appended to V; normalization via fast reciprocal + PE
    outer-product broadcast (kept f32r in all modes)
  - V computed in natural [S, H*HS] layout; O^T[e, s] comes out of AV
    directly in the layout the output projection needs as stationary.
"""

import sys

sys.path.insert(0, "/opt/trn_rl_repo")

import numpy as np

B, S, E = 8, 1024, 1024
H, HS = 16, 64
N_CORES = 8
P = 128  # partitions
CH = 512  # matmul free-dim chunk
ET = E // P  # 8 e-tiles
ST_ = S // P  # 8 s/t-tiles
NCH = S // CH  # 2 s-chunks

MODE = "f32r"  # "f32r" | "bf16"

_cached = {}


def _np_mm_dtype():
    if MODE == "bf16":
        import ml_dtypes

        return ml_dtypes.bfloat16
    return np.float32


def _build(reps=1):
    import concourse.mybir as mybir
    import concourse.tile as tile
    from concourse import bacc

    F32 = mybir.dt.float32
    F32R = mybir.dt.float32r
    DT = mybir.dt.bfloat16 if MODE == "bf16" else F32R
    EXP = mybir.ActivationFunctionType.Exp

    nc = bacc.Bacc("TRN2", target_bir_lowering=False, debug=False, num_devices=N_CORES)

    xT = nc.dram_tensor("xT", [E, S], DT, kind="ExternalInput").ap()
    wq = nc.dram_tensor("wq", [E, E], DT, kind="ExternalInput").ap()
    wk = nc.dram_tensor("wk", [E, E], DT, kind="ExternalInput").ap()
    wv = nc.dram_tensor("wv", [E, E], DT, kind="ExternalInput").ap()
    wpT = nc.dram_tensor("wpT", [E, E], DT, kind="ExternalInput").ap()
    bias128 = nc.dram_tensor("bias128", [P, E], F32, kind="ExternalInput").ap()
    mask = nc.dram_tensor("mask", [P, P], DT, kind="ExternalInput").ap()
    ones1 = nc.dram_tensor("ones1", [1, HS], F32R, kind="ExternalInput").ap()
    onescols = nc.dram_tensor("onescols", [P, H], DT, kind="ExternalInput").ap()
    out = nc.dram_tensor("out", [S, E], F32, kind="ExternalOutput").ap()

    VW = HS + 1  # V columns per head incl ones column

    with tile.TileContext(nc) as tc:
      for _rep in range(reps):
        with (
            tc.tile_pool(name="qt_pool", bufs=1) as qtp,
            tc.tile_pool(name="kt_pool", bufs=1) as ktp,
            tc.tile_pool(name="va_pool", bufs=1) as vap,
            tc.tile_pool(name="ot_pool", bufs=1) as otp,
            tc.tile_pool(name="const_pool", bufs=1) as cp,
        ):
            qt = [qtp.tile([P, S], DT, name=f"qt{m}") for m in range(ET)]
            kt = [ktp.tile([P, S], DT, name=f"kt{m}") for m in range(ET)]
            vaug = [vap.tile([P, H * VW], DT, name=f"vaug{t}") for t in range(ST_)]
            ot = [otp.tile([P, S], DT, name=f"ot{m}") for m in range(ET)]
            mask_t = cp.tile([P, P], DT, name="mask_t")
            ones1_t = cp.tile([1, HS], F32R, name="ones1_t")
            bias_t = cp.tile([P, E], F32, name="bias_t")
            nc.sync.dma_start(mask_t[:], mask[:])
            nc.sync.dma_start(ones1_t[:], ones1[:])
            nc.sync.dma_start(bias_t[:], bias128[:])

            # ---------------- Phase 1: Q^T, K^T, V ----------------
            with (
                tc.tile_pool(name="xT_pool", bufs=1) as xtp,
                tc.tile_pool(name="w_pool", bufs=12) as wp,
                tc.tile_pool(name="ps1", bufs=4, space="PSUM") as ps1,
            ):
                xt = [xtp.tile([P, S], DT, name=f"xt{e}") for e in range(ET)]
                for e in range(ET):
                    nc.sync.dma_start(xt[e][:], xT[e * P:(e + 1) * P, :])

                # Q^T and K^T: [E, S], heads stacked along partition rows
                for w_ap, dst, pfx in ((wq, qt, "q"), (wk, kt, "k")):
                    for m in range(ET):
                        wt = [
                            wp.tile([P, P], DT, tag="w", name=f"w{pfx}{m}_{e}")
                            for e in range(ET)
                        ]
                        for e in range(ET):
                            nc.sync.dma_start(
                                wt[e][:], w_ap[e * P:(e + 1) * P, m * P:(m + 1) * P]
                            )
                        for c in range(NCH):
                            ps = ps1.tile([P, CH], F32, tag="p1", name=f"ps{pfx}{m}_{c}")
                            for e in range(ET):
                                nc.tensor.matmul(
                                    ps[:],
                                    wt[e][:],
                                    xt[e][:, c * CH:(c + 1) * CH],
                                    start=(e == 0),
                                    stop=(e == ET - 1),
                                )
                            nc.vector.tensor_copy(dst[m][:, c * CH:(c + 1) * CH], ps[:])

                # ones columns of V_aug (DMA'd from host const)
                for t in range(ST_):
                    onesdst = vaug[t].rearrange("p (h d) -> p h d", d=VW)[:, :, HS:VW]
                    nc.sync.dma_start(onesdst, onescols[:].unsqueeze(2))

                # V natural [S, H*HS], scattered into vaug with stride VW
                for t in range(ST_):
                    wvt = [
                        wp.tile([P, CH], DT, tag="wv", name=f"wv{t}_{c}_{e}")
                        for c in range(NCH)
                        for e in range(ET)
                    ]
                    for c in range(NCH):
                        for e in range(ET):
                            nc.sync.dma_start(
                                wvt[c * ET + e][:],
                                wv[e * P:(e + 1) * P, c * CH:(c + 1) * CH],
                            )
                        ps = ps1.tile([P, CH], F32, tag="p1", name=f"psv{t}_{c}")
                        for e in range(ET):
                            nc.tensor.matmul(
                                ps[:],
                                xt[e][:, t * P:(t + 1) * P],
                                wvt[c * ET + e][:],
                                start=(e == 0),
                                stop=(e == ET - 1),
                            )
                        nheads = CH // HS  # heads covered by this chunk
                        src = ps.rearrange("p (h d) -> p h d", d=HS)
                        dstcols = vaug[t][:, c * nheads * VW:(c + 1) * nheads * VW]
                        dst3 = dstcols.rearrange("p (h d) -> p h d", d=VW)[:, :, 0:HS]
                        nc.vector.tensor_copy(dst3, src)

            # ---------------- Phase 2+3: scores^T, softmax, AV ----------------
            with (
                tc.tile_pool(name="a_pool", bufs=6) as apl,
                tc.tile_pool(name="r_pool", bufs=4) as rpl,
                tc.tile_pool(name="st_ps", bufs=3, space="PSUM") as stps,
                tc.tile_pool(name="av_ps", bufs=3, space="PSUM") as avps,
                tc.tile_pool(name="bc_ps", bufs=2, space="PSUM") as bcps,
            ):
                for j in range(NCH):
                    s0 = j * CH
                    for h in range(H):
                        mt = h // 2
                        hoff = (h % 2) * HS
                        n_t = (s0 + CH) // P  # t-tiles 0 .. n_t-1
                        av = avps.tile([VW, CH], F32, tag="av", name=f"av{j}_{h}")
                        for i in range(n_t):
                            # cols [0, pfx) of this block have s < t: causally
                            # masked. Never compute or read them.
                            pfx = i * P - s0
                            vs = max(pfx, 0)
                            st = stps.tile([P, CH], F32, tag="st", name=f"st{j}_{h}_{i}")
                            nc.tensor.matmul(
                                st[:, vs:CH],
                                kt[mt][hoff:hoff + HS, i * P:(i + 1) * P],
                                qt[mt][hoff:hoff + HS, s0 + vs:s0 + CH],
                                start=True,
                                stop=True,
                            )
                            at = apl.tile([P, CH], DT, tag="a", name=f"a{j}_{h}_{i}")
                            nc.scalar.activation(
                                at[:, vs:CH], st[:, vs:CH], EXP, scale=float(HS) ** -0.5
                            )
                            if 0 <= pfx <= CH - P:
                                # diagonal block: keep only s >= t
                                nc.vector.tensor_mul(
                                    at[:, pfx:pfx + P], at[:, pfx:pfx + P], mask_t[:]
                                )
                            nc.tensor.matmul(
                                av[:, vs:CH],
                                vaug[i][:, h * VW:(h + 1) * VW],
                                at[:, vs:CH],
                                start=(i == 0),
                                stop=(i == n_t - 1),
                            )
                        # normalize: row VW-1 of av is the softmax denominator
                        rs = rpl.tile([1, CH], F32, tag="rs", name=f"rs{j}_{h}")
                        nc.vector.tensor_copy(rs[:], av[HS:VW, :])
                        rf = rpl.tile([1, CH], F32, tag="rf", name=f"rf{j}_{h}")
                        nc.vector.reciprocal_approx_fast(rf[:], rs[:])
                        rinv = rpl.tile([1, CH], F32R, tag="r", name=f"r{j}_{h}")
                        nc.vector.tensor_copy(rinv[:], rf[:])
                        bc = bcps.tile([HS, CH], F32, tag="bc", name=f"bc{j}_{h}")
                        nc.tensor.matmul(bc[:], ones1_t[:], rinv[:], start=True, stop=True)
                        rb = rpl.tile([HS, CH], F32, tag="rb", name=f"rb{j}_{h}")
                        nc.vector.tensor_copy(rb[:], bc[:])
                        nc.vector.tensor_mul(
                            ot[mt][hoff:hoff + HS, s0:s0 + CH], av[0:HS, :], rb[:]
                        )

            # ---------------- Phase 4: output projection ----------------
            with (
                tc.tile_pool(name="wp_pool", bufs=1) as wpp,
                tc.tile_pool(name="fo_pool", bufs=4) as fop,
                tc.tile_pool(name="ps4", bufs=4, space="PSUM") as ps4,
            ):
                wpt = [wpp.tile([P, E], DT, name=f"wpt{e}") for e in range(ET)]
                for e in range(ET):
                    nc.sync.dma_start(wpt[e][:], wpT[e * P:(e + 1) * P, :])
                for m in range(ST_):
                    for c in range(NCH):
                        ps = ps4.tile([P, CH], F32, tag="p4", name=f"pso{m}_{c}")
                        for e in range(ET):
                            nc.tensor.matmul(
                                ps[:],
                                ot[e][:, m * P:(m + 1) * P],
                                wpt[e][:, c * CH:(c + 1) * CH],
                                start=(e == 0),
                                stop=(e == ET - 1),
                            )
                        fo = fop.tile([P, CH], F32, tag="fo", name=f"fo{m}_{c}")
                        # bias add fused into the PSUM->SBUF eviction
                        nc.vector.tensor_add(fo[:], ps[:], bias_t[:, c * CH:(c + 1) * CH])
                        nc.sync.dma_start(out[m * P:(m + 1) * P, c * CH:(c + 1) * CH], fo[:])

    nc.compile()
    return nc


def _get_nc():
    if "nc" not in _cached:
        _cached["nc"] = _build()
    return _cached["nc"]


def make_in_maps(x, Wq, Wk, Wv, Wproj, bproj):
    mdt = _np_mm_dtype()
    x = np.ascontiguousarray(x, dtype=np.float32)
    wq_f = np.ascontiguousarray(Wq.transpose(1, 0, 2).reshape(E, E)).astype(mdt)
    wk_f = np.ascontiguousarray(Wk.transpose(1, 0, 2).reshape(E, E)).astype(mdt)
    wv_f = np.ascontiguousarray(Wv.transpose(1, 0, 2).reshape(E, E)).astype(mdt)
    wpT = np.ascontiguousarray(np.asarray(Wproj).T).astype(mdt)
    bias128 = np.broadcast_to(
        np.asarray(bproj, dtype=np.float32).reshape(1, E), (P, E)
    ).copy()
    shared = {
        "wq": wq_f, "wk": wk_f, "wv": wv_f, "wpT": wpT, "bias128": bias128,
        "mask": np.triu(np.ones((P, P))).astype(mdt),  # [t, s]: 1 where s >= t
        "ones1": np.ones((1, HS), dtype=np.float32),
        "onescols": np.ones((P, H)).astype(mdt),
    }
    return [
        {"xT": np.ascontiguousarray(x[b].T).astype(mdt), **shared}
        for b in range(N_CORES)
    ]


def kernel(x, Wq, Wk, Wv, Wproj, bproj):
    from concourse.bass_utils import run_bass_kernel_spmd

    nc = _get_nc()
    in_maps = make_in_maps(x, Wq, Wk, Wv, Wproj, bproj)
    res = run_bass_kernel_spmd(nc, in_maps, core_ids=list(range(N_CORES)))
    return np.stack([res.results[b]["out"] for b in range(B)], axis=0)


# revision 17
# speedup vs baseline: 1.5824x; 1.2799x over previous
"""Multi-head causal attention (B=8, S=1024, E=1024, H=16, HS=64) on 8 TRN2 cores.

Strategy: pure data parallel over batch -- each NeuronCore computes one batch
element end to end (QKV projections, causal attention, output projection);
no collectives. Matmul operands are float32r (fast-path fp32, ~2 cyc/row) or
bfloat16 (1 cyc/row) per MODE, with layouts chosen so no on-chip transposes
are needed:
  - host feeds x[b].T, per-head-flattened + pre-tiled W, Wproj.T, and a
    pre-broadcast bias
  - Q^T, K^T computed as [E, S] (heads stacked along partitions)
  - scores computed transposed: S^T[t, s] = K_h^T(t-block) x Q_h^T
  - causally-masked column ranges are never computed (PSUM subrange accum)
  - softmax: exp on ACT; row sums fused into the AV matmul via a ones
    column appended to V
  - normalization is batched per s-chunk (off the PE critical path): one
    fast reciprocal over all 16 head denominators, pair-wise PE
    outer-product broadcasts, DVE multiplies (kept f32r in all modes)
  - V computed in natural [S, H*HS] layout; O^T[e, s] comes out of AV
    directly in the layout the output projection needs as stationary.
"""

import sys

sys.path.insert(0, "/opt/trn_rl_repo")

import numpy as np

B, S, E = 8, 1024, 1024
H, HS = 16, 64
N_CORES = 8
P = 128  # partitions
CH = 512  # matmul free-dim chunk
ET = E // P  # 8 e-tiles
ST_ = S // P  # 8 s/t-tiles
NCH = S // CH  # 2 s-chunks

MODE = "f32r"  # "f32r" | "bf16"

_cached = {}


def _np_mm_dtype():
    if MODE == "bf16":
        import ml_dtypes

        return ml_dtypes.bfloat16
    return np.float32


def _build(reps=1):
    import concourse.mybir as mybir
    import concourse.tile as tile
    from concourse import bacc

    F32 = mybir.dt.float32
    F32R = mybir.dt.float32r
    DT = mybir.dt.bfloat16 if MODE == "bf16" else F32R
    EXP = mybir.ActivationFunctionType.Exp

    nc = bacc.Bacc("TRN2", target_bir_lowering=False, debug=False, num_devices=N_CORES)

    xT = nc.dram_tensor("xT", [E, S], DT, kind="ExternalInput").ap()
    # wq/wk pre-tiled on host: [m, p, e*128+f] so each m-block loads as one
    # contiguous [128, E] DMA whose [:, e*128:(e+1)*128] slice is the
    # stationary tile for (e, m)
    wq = nc.dram_tensor("wq", [ET, P, E], DT, kind="ExternalInput").ap()
    wk = nc.dram_tensor("wk", [ET, P, E], DT, kind="ExternalInput").ap()
    wv = nc.dram_tensor("wv", [E, E], DT, kind="ExternalInput").ap()
    wpT = nc.dram_tensor("wpT", [E, E], DT, kind="ExternalInput").ap()
    bias128 = nc.dram_tensor("bias128", [P, E], F32, kind="ExternalInput").ap()
    mask = nc.dram_tensor("mask", [P, P], DT, kind="ExternalInput").ap()
    ones1 = nc.dram_tensor("ones1", [1, HS], F32R, kind="ExternalInput").ap()
    onescols = nc.dram_tensor("onescols", [P, H], DT, kind="ExternalInput").ap()
    out = nc.dram_tensor("out", [S, E], F32, kind="ExternalOutput").ap()

    VW = HS + 1  # V columns per head incl ones column

    with tile.TileContext(nc) as tc:
      for _rep in range(reps):
        with (
            tc.tile_pool(name="qt_pool", bufs=1) as qtp,
            tc.tile_pool(name="kt_pool", bufs=1) as ktp,
            tc.tile_pool(name="va_pool", bufs=1) as vap,
            tc.tile_pool(name="ot_pool", bufs=1) as otp,
            tc.tile_pool(name="const_pool", bufs=1) as cp,
        ):
            qt = [qtp.tile([P, S], DT, name=f"qt{m}") for m in range(ET)]
            kt = [ktp.tile([P, S], DT, name=f"kt{m}") for m in range(ET)]
            vaug = [vap.tile([P, H * VW], DT, name=f"vaug{t}") for t in range(ST_)]
            ot = [otp.tile([P, S], DT, name=f"ot{m}") for m in range(ET)]
            mask_t = cp.tile([P, P], DT, name="mask_t")
            ones1_t = cp.tile([1, HS], F32R, name="ones1_t")
            bias_t = cp.tile([P, E], F32, name="bias_t")
            nc.sync.dma_start(mask_t[:], mask[:])
            nc.sync.dma_start(ones1_t[:], ones1[:])
            nc.sync.dma_start(bias_t[:], bias128[:])

            # ---------------- Phase 1: Q^T, K^T, V ----------------
            with (
                tc.tile_pool(name="xT_pool", bufs=1) as xtp,
                tc.tile_pool(name="w_pool", bufs=3) as wp,
                tc.tile_pool(name="wv_pool", bufs=1) as wvp,
                tc.tile_pool(name="ps1", bufs=4, space="PSUM") as ps1,
            ):
                xt = [xtp.tile([P, S], DT, name=f"xt{e}") for e in range(ET)]
                for e in range(ET):
                    nc.sync.dma_start(xt[e][:], xT[e * P:(e + 1) * P, :])

                # Q^T and K^T: [E, S], heads stacked along partition rows
                for w_ap, dst, pfx in ((wq, qt, "q"), (wk, kt, "k")):
                    for m in range(ET):
                        wt = wp.tile([P, E], DT, tag="w", name=f"w{pfx}{m}")
                        nc.sync.dma_start(wt[:], w_ap[m])
                        for c in range(NCH):
                            ps = ps1.tile([P, CH], F32, tag="p1", name=f"ps{pfx}{m}_{c}")
                            for e in range(ET):
                                nc.tensor.matmul(
                                    ps[:],
                                    wt[:, e * P:(e + 1) * P],
                                    xt[e][:, c * CH:(c + 1) * CH],
                                    start=(e == 0),
                                    stop=(e == ET - 1),
                                )
                            nc.vector.tensor_copy(dst[m][:, c * CH:(c + 1) * CH], ps[:])

                # ones columns of V_aug (DMA'd from host const)
                for t in range(ST_):
                    onesdst = vaug[t].rearrange("p (h d) -> p h d", d=VW)[:, :, HS:VW]
                    nc.sync.dma_start(onesdst, onescols[:].unsqueeze(2))

                # V natural [S, H*HS], scattered into vaug with stride VW
                nheads = CH // HS  # heads covered per chunk
                for c in range(NCH):
                    wvt = [
                        wvp.tile([P, CH], DT, tag=f"wv{e}", name=f"wv{c}_{e}")
                        for e in range(ET)
                    ]
                    for e in range(ET):
                        nc.sync.dma_start(
                            wvt[e][:], wv[e * P:(e + 1) * P, c * CH:(c + 1) * CH]
                        )
                    for t in range(ST_):
                        ps = ps1.tile([P, CH], F32, tag="p1", name=f"psv{t}_{c}")
                        for e in range(ET):
                            nc.tensor.matmul(
                                ps[:],
                                xt[e][:, t * P:(t + 1) * P],
                                wvt[e][:],
                                start=(e == 0),
                                stop=(e == ET - 1),
                            )
                        src = ps.rearrange("p (h d) -> p h d", d=HS)
                        dstcols = vaug[t][:, c * nheads * VW:(c + 1) * nheads * VW]
                        dst3 = dstcols.rearrange("p (h d) -> p h d", d=VW)[:, :, 0:HS]
                        nc.vector.tensor_copy(dst3, src)

            # ---------------- Phase 2+3: scores^T, softmax, AV ----------------
            with (
                tc.tile_pool(name="a_pool", bufs=5) as apl,
                tc.tile_pool(name="avsb_pool", bufs=1) as avsbp,
                tc.tile_pool(name="r_pool", bufs=4) as rpl,
                tc.tile_pool(name="st_ps", bufs=3, space="PSUM") as stps,
                tc.tile_pool(name="av_ps", bufs=3, space="PSUM") as avps,
                tc.tile_pool(name="bc_ps", bufs=2, space="PSUM") as bcps,
            ):
                for j in range(NCH):
                    s0 = j * CH
                    n_t = (s0 + CH) // P  # t-tiles 0 .. n_t-1
                    avsb = [
                        avsbp.tile([VW, CH], F32, tag=f"avsb{h}", name=f"avsb{j}_{h}")
                        for h in range(H)
                    ]
                    for h in range(H):
                        mt = h // 2
                        hoff = (h % 2) * HS
                        av = avps.tile([VW, CH], F32, tag="av", name=f"av{j}_{h}")
                        for i in range(n_t):
                            # cols [0, pfx) of this block have s < t: causally
                            # masked. Never compute or read them.
                            pfx = i * P - s0
                            vs = max(pfx, 0)
                            st = stps.tile([P, CH], F32, tag="st", name=f"st{j}_{h}_{i}")
                            nc.tensor.matmul(
                                st[:, vs:CH],
                                kt[mt][hoff:hoff + HS, i * P:(i + 1) * P],
                                qt[mt][hoff:hoff + HS, s0 + vs:s0 + CH],
                                start=True,
                                stop=True,
                            )
                            at = apl.tile([P, CH], DT, tag="a", name=f"a{j}_{h}_{i}")
                            nc.scalar.activation(
                                at[:, vs:CH], st[:, vs:CH], EXP, scale=float(HS) ** -0.5
                            )
                            if 0 <= pfx <= CH - P:
                                # diagonal block: keep only s >= t
                                nc.vector.tensor_mul(
                                    at[:, pfx:pfx + P], at[:, pfx:pfx + P], mask_t[:]
                                )
                            nc.tensor.matmul(
                                av[:, vs:CH],
                                vaug[i][:, h * VW:(h + 1) * VW],
                                at[:, vs:CH],
                                start=(i == 0),
                                stop=(i == n_t - 1),
                            )
                        # evict accumulated AV (+denominator row) to SBUF
                        nc.vector.tensor_copy(avsb[h][:], av[:])

                    # batched normalization for all 16 heads of this chunk
                    for h in range(H):
                        mt = h // 2
                        hoff = (h % 2) * HS
                        rs = rpl.tile([1, CH], F32, tag="rs", name=f"rs{j}_{h}")
                        nc.vector.tensor_copy(rs[:], avsb[h][HS:VW, :])
                        rf = rpl.tile([1, CH], F32, tag="rf", name=f"rf{j}_{h}")
                        nc.vector.reciprocal_approx_fast(rf[:], rs[:])
                        rr = rpl.tile([1, CH], F32R, tag="rr", name=f"rr{j}_{h}")
                        nc.vector.tensor_copy(rr[:], rf[:])
                        bc = bcps.tile([HS, CH], F32, tag="bc", name=f"bc{j}_{h}")
                        nc.tensor.matmul(bc[:], ones1_t[:], rr[:], start=True, stop=True)
                        nc.vector.tensor_mul(
                            ot[mt][hoff:hoff + HS, s0:s0 + CH],
                            avsb[h][0:HS, :],
                            bc[:],
                        )

            # ---------------- Phase 4: output projection ----------------
            with (
                tc.tile_pool(name="wp_pool", bufs=1) as wpp,
                tc.tile_pool(name="fo_pool", bufs=4) as fop,
                tc.tile_pool(name="ps4", bufs=4, space="PSUM") as ps4,
            ):
                wpt = [wpp.tile([P, E], DT, name=f"wpt{e}") for e in range(ET)]
                for e in range(ET):
                    nc.sync.dma_start(wpt[e][:], wpT[e * P:(e + 1) * P, :])
                for m in range(ST_):
                    for c in range(NCH):
                        ps = ps4.tile([P, CH], F32, tag="p4", name=f"pso{m}_{c}")
                        for e in range(ET):
                            nc.tensor.matmul(
                                ps[:],
                                ot[e][:, m * P:(m + 1) * P],
                                wpt[e][:, c * CH:(c + 1) * CH],
                                start=(e == 0),
                                stop=(e == ET - 1),
                            )
                        fo = fop.tile([P, CH], F32, tag="fo", name=f"fo{m}_{c}")
                        # bias add fused into the PSUM->SBUF eviction
                        nc.vector.tensor_add(fo[:], ps[:], bias_t[:, c * CH:(c + 1) * CH])
                        nc.sync.dma_start(out[m * P:(m + 1) * P, c * CH:(c + 1) * CH], fo[:])

    nc.compile()
    return nc


def _get_nc():
    if "nc" not in _cached:
        _cached["nc"] = _build()
    return _cached["nc"]


def _pretile(w_flat, mdt):
    # [E, E] -> [m, p, e*128+f]: tile (e, m) of the stationary operand is
    # w_flat[e*128+p, m*128+f] stored contiguously per m-block
    return np.ascontiguousarray(
        w_flat.reshape(ET, P, ET, P).transpose(2, 1, 0, 3).reshape(ET, P, E)
    ).astype(mdt)


def make_in_maps(x, Wq, Wk, Wv, Wproj, bproj):
    mdt = _np_mm_dtype()
    x = np.ascontiguousarray(x, dtype=np.float32)
    wq_f = np.ascontiguousarray(Wq.transpose(1, 0, 2).reshape(E, E)).astype(np.float32)
    wk_f = np.ascontiguousarray(Wk.transpose(1, 0, 2).reshape(E, E)).astype(np.float32)
    wv_f = np.ascontiguousarray(Wv.transpose(1, 0, 2).reshape(E, E)).astype(mdt)
    wpT = np.ascontiguousarray(np.asarray(Wproj).T).astype(mdt)
    bias128 = np.broadcast_to(
        np.asarray(bproj, dtype=np.float32).reshape(1, E), (P, E)
    ).copy()
    shared = {
        "wq": _pretile(wq_f, mdt), "wk": _pretile(wk_f, mdt),
        "wv": wv_f, "wpT": wpT, "bias128": bias128,
        "mask": np.triu(np.ones((P, P))).astype(mdt),  # [t, s]: 1 where s >= t
        "ones1": np.ones((1, HS), dtype=np.float32),
        "onescols": np.ones((P, H)).astype(mdt),
    }
    return [
        {"xT": np.ascontiguousarray(x[b].T).astype(mdt), **shared}
        for b in range(N_CORES)
    ]


def kernel(x, Wq, Wk, Wv, Wproj, bproj):
    from concourse.bass_utils import run_bass_kernel_spmd

    nc = _get_nc()
    in_maps = make_in_maps(x, Wq, Wk, Wv, Wproj, bproj)
    res = run_bass_kernel_spmd(nc, in_maps, core_ids=list(range(N_CORES)))
    return np.stack([res.results[b]["out"] for b in range(B)], axis=0)
